# revision 43
# baseline (speedup 1.0000x reference)
"""Trainium2 Bass kernel for nn_ATConv (dynamic per-(b,c) 3x3 depthwise conv
between two 1x1 convs, with a pooled-gelu kernel-generation branch).

Sharding: data-parallel over batch B=16 across 8 NeuronCores (2 images/core).
Each core processes its 2 images as 3 "planes" of 128 partitions:
  P0 = img0 channels 0:128, P1 = img1 channels 0:128,
  P2 = packed [img0 c128:192 | img1 c128:192].

v3 design (~172us vs the 205us v2 baseline):
  - Single TileContext region, one rotating PSUM pool (3 x [128,1024] +
    a small kgen bank): all phases share PSUM, so the tile scheduler can
    overlap them freely instead of hitting pool-scope barriers.
  - Input DMA is piece-streamed (xa0 on sync in 6 pieces, xb on gpsimd,
    xa1 queued behind both) and phase A consumes it per-range, so the
    8us DMA head and ~25us load window overlap the first compute.
  - Weights ride the scalar queue in parallel with the first x pieces.
  - Pooling for kernel-gen is a fp16 tree-reduce on DVE (4 halvings +
    tensor_reduce), ~6us/plane instead of ~11us of 1x copies.
  - kgen0 needs only pools of x -> runs before A(P2); kgen1 needs only
    pools + g16[0] -> emitted mid-A(P2).  Both kernel-gen chains use
    spare columns of a dedicated 1-bank PSUM tile.
  - Phase C is split DVE-rows/PE-rows per plane (RD):  DVE taps use
    tensor_scalar 4x + tensor_tensor 2x with an element-shifted copy
    (xpo) for odd column shifts; dw=+1 taps use row-clipped 2D APs (no
    fixup), dw=-1 taps keep flat reads + tiny column fixups.  PE taps
    are diagonal matmuls with row/col-clipped 2D moving APs (fixup
    free), evicted by ACT.
  - C planes run P2 -> P1 -> P0 on DVE (P2 gates every output stream);
    phase D output streams are emitted at their earliest unlock points
    (o0 hi-ranges between c_pe(P0) and c_pe(P1), etc.) with hi rows
    first so D overlaps the DVE tail.
  - xpo copies ride the idle sync queue in row-pieces so c_dve blocks
    only wait for their own rows.
  - Output stores are 2-range (4KB/row) DMAs spread over sync, gpsimd
    and scalar queues.
"""
import numpy as np

import concourse.bacc as bacc
import concourse.mybir as mybir
import concourse.tile as tile
from concourse import bass_utils

dt = mybir.dt
Alu = mybir.AluOpType
Act = mybir.ActivationFunctionType
Ax = mybir.AxisListType

B, C, H, W = 16, 192, 96, 96
L = H * W            # 9216
K2 = 9
SEG = L // K2        # 1024
NCORES = 8
NR = 9               # ranges of SEG
RT = 1024
INV_SQRT2 = float(1.0 / np.sqrt(2.0))

# tap index t = 3*(dh+1) + (dw+1); center tap = 4
TAPS = [(t // 3 - 1, t % 3 - 1) for t in range(9)]
# rows [0, RD[p]) of plane p run on DVE; the rest on PE (diag matmuls)
RD = {"P0": 48, "P1": 24, "P2": 50}
RDMAX = max(RD.values())
XPO_SZ = (RDMAX + 1) * W + 2
# ranges whose phase-A eviction runs on DVE instead of ACT (per plane)
EV_DVE = set()
# taps whose scale-copy runs on ACT (per plane); adds stay on DVE
ACT_TAPS = {"P0": (), "P1": (), "P2": ()}
# DVE tap row-block size (bounds the tap scratch tile)
TBLK = 33

_BUILT = {}


def build():
    nc = bacc.Bacc("TRN2", target_bir_lowering=False, debug=False,
                   num_devices=NCORES)

    f16, f32 = dt.float16, dt.float32
    x0 = nc.dram_tensor("x0", [C, L], f16, kind="ExternalInput").ap()
    x1 = nc.dram_tensor("x1", [C, L], f16, kind="ExternalInput").ap()
    wbig = nc.dram_tensor("wbig", [128, 1408], f16, kind="ExternalInput").ap()
    f32b = nc.dram_tensor("f32b", [128, 13], f32, kind="ExternalInput").ap()
    bk9 = nc.dram_tensor("bk9", [9, 192], f32, kind="ExternalInput").ap()
    wg2 = nc.dram_tensor("wg2", [9, 9], f16, kind="ExternalInput").ap()
    out0 = nc.dram_tensor("out0", [C, L], f16, kind="ExternalOutput").ap()
    out1 = nc.dram_tensor("out1", [C, L], f16, kind="ExternalOutput").ap()

    PL = ["P0", "P1", "P2"]

    with tile.TileContext(nc) as tc:
        with tc.tile_pool(name="wpool", bufs=1) as wp, \
             tc.tile_pool(name="xin", bufs=1) as xin, \
             tc.tile_pool(name="small", bufs=1) as sm, \
             tc.tile_pool(name="xppool", bufs=1) as xpp, \
             tc.tile_pool(name="ypool", bufs=1) as yp, \
             tc.tile_pool(name="xopool", bufs=2) as xop, \
             tc.tile_pool(name="tspool", bufs=2) as tsp, \
             tc.tile_pool(name="dg", bufs=2) as dg, \
             tc.tile_pool(name="stage", bufs=3) as stg, \
             tc.tile_pool(name="ps", bufs=3, space="PSUM") as ps, \
             tc.tile_pool(name="psk", bufs=1, space="PSUM") as psk:

            # ---- persistent weights / biases (front of the sync queue) ----
            wbig_t = wp.tile([128, 1408], f16, tag="wbig", name="wbig_t")
            nc.scalar.dma_start(wbig_t[:], wbig[:, :])
            f32b_t = wp.tile([128, 13], f32, tag="f32b", name="f32b_t")
            nc.scalar.dma_start(f32b_t[:], f32b[:, :])
            bkb = wp.tile([9, 192], f32, tag="bkb", name="bkb")
            nc.scalar.dma_start(bkb[:], bk9[:, :])
            wgt = wp.tile([9, 9], f16, tag="wgt", name="wgt")
            nc.scalar.dma_start(wgt[:], wg2[:, :])

            # ---- input x tiles ----
            # xa0 (sync) and xb (gpsimd) stream first -- they feed phase
            # A(P0).  xa1's DMAs are issued from the vector queue, gated
            # behind the first pool-tree op, so its transfers don't steal
            # bandwidth from the critical xa0/xb head.
            xa0 = xin.tile([128, L], f16, tag="xa0", name="xa0")
            xa1 = xin.tile([128, L], f16, tag="xa1", name="xa1")
            xb = xin.tile([128, L], f16, tag="xb", name="xb")
            Q = L // 3
            for lo_, hi_ in [(0, 1024), (1024, 2048), (2048, 3072),
                             (3072, 4608), (4608, 6656), (6656, L)]:
                nc.sync.dma_start(xa0[:, lo_:hi_], x0[0:128, lo_:hi_])
            for qi in range(3):
                lo_, hi_ = qi * Q, (qi + 1) * Q
                nc.gpsimd.dma_start(xb[0:64, lo_:hi_], x0[128:192, lo_:hi_])
                nc.gpsimd.dma_start(xb[64:128, lo_:hi_], x1[128:192, lo_:hi_])

            def xa1_load(qi):
                # behind xa0 on sync / behind xb on gpsimd, so the xa1
                # transfers start only once the phase-A(P0) inputs are in
                lo_, hi_ = qi * Q, (qi + 1) * Q
                eng = nc.sync if qi == 0 else nc.gpsimd
                eng.dma_start(xa1[:, lo_:hi_], x1[0:128, lo_:hi_])
            wt = {}
            _off = 0
            for nm, cols in [("a1", 128), ("a2", 128), ("a3", 64),
                             ("a4", 128), ("p1", 128), ("p2", 128),
                             ("p3", 64), ("p4", 128)]:
                wt[nm] = wbig_t[:, _off:_off + cols]
                _off += cols
            wka = wbig_t[:, _off:_off + 192]; _off += 192
            wkbp = wbig_t[:, _off:_off + 192]; _off += 192
            ident_sb = wbig_t[:, _off:_off + 128]; _off += 128
            bias = {"bx_a": f32b_t[:, 0:1], "bx_b": f32b_t[:, 1:2],
                    "dc_a": f32b_t[:, 2:3], "dc_b": f32b_t[:, 3:4]}
            bgb = f32b_t[:, 4:13]
            biasx = {"P0": bias["bx_a"], "P1": bias["bx_a"],
                     "P2": bias["bx_b"]}

            factor = {}
            for p, srcn in [("P0", "dc_a"), ("P2", "dc_b")]:
                f = sm.tile([128, 1], f32, tag=f"factor{p}", name=f"factor{p}")
                nc.scalar.activation(f[:], bias[srcn], Act.Sigmoid,
                                     scale=1.0, bias=0.0)
                f9 = sm.tile([128, 1], f32, tag=f"f9{p}", name=f"f9{p}")
                nc.vector.tensor_scalar(f9[:], f[:], 1.0 / 9, None, Alu.mult)
                factor[p] = f9
            factor["P1"] = factor["P0"]

            # xpe tiles carry 2 zero guard elements on each side (data at
            # offset 2) so shifted reads at the plane edges see zeros.
            xpe = {p: xpp.tile([128, L + 4], f16, tag=f"xpe{p}",
                               name=f"xpe{p}")
                   for p in PL}
            for p in PL:
                nc.vector.memset(xpe[p][:, 0:2], 0.0)
                nc.vector.memset(xpe[p][:, L + 2:L + 4], 0.0)

            pool = {p: sm.tile([128, 9], f32, tag=f"pool{p}",
                               name=f"pool{p}")
                    for p in PL}
            pool16 = {p: sm.tile([128, 9], f16, tag=f"pool16{p}",
                                 name=f"pool16{p}")
                      for p in PL}

            y_store = {"P0": yp.tile([128, L], f16, tag="yP0", name="yP0")}

            kfin = {}
            kneg = {}
            g16 = {}
            diag = {}

            def xd(p):
                return xpe[p][:, 2:2 + L]

            # ---------- pooling: fp16 tree reduce on DVE ----------
            # scratch: P0/P2 borrow yP0 (only written later, in phase C);
            # P1 borrows xpe[P1] (written later by A(P1) evictions).
            def pool_tree(p, xt, scr_base, after_lvl0=None):
                s1 = scr_base[:, 0:4608]
                s2 = scr_base[:, 4608:6912]
                for qi in range(3):
                    src = xt[:, qi * 3072:(qi + 1) * 3072].rearrange(
                        "c (s h) -> c s h", s=3)
                    d = s1[:, qi * 1536:(qi + 1) * 1536].rearrange(
                        "c (s h) -> c s h", s=3)
                    nc.vector.tensor_tensor(d, src[:, :, 0:512],
                                            src[:, :, 512:1024], Alu.add)
                    if after_lvl0 is not None:
                        after_lvl0(qi)
                s1v = s1.rearrange("c (s h) -> c s h", s=9)      # [c,9,512]
                s2v = s2.rearrange("c (s h) -> c s h", s=9)      # [c,9,256]
                nc.vector.tensor_tensor(s2v, s1v[:, :, 0:256],
                                        s1v[:, :, 256:512], Alu.add)
                d2 = s1[:, 0:1152].rearrange("c (s h) -> c s h", s=9)
                nc.vector.tensor_tensor(d2, s2v[:, :, 0:128],
                                        s2v[:, :, 128:256], Alu.add)
                d3 = s2[:, 0:576].rearrange("c (s h) -> c s h", s=9)
                nc.vector.tensor_tensor(d3, d2[:, :, 0:64],
                                        d2[:, :, 64:128], Alu.add)
                nc.vector.tensor_reduce(
                    pool[p][:].rearrange("c (s o) -> c s o", s=9), d3,
                    Ax.X, Alu.add)
                nc.vector.tensor_scalar(pool16[p][:], pool[p][:],
                                        1.0 / SEG, None, Alu.mult)

            # ---------- phase A: xp = Wx x + bx ----------
            def a_range(p, r):
                l0 = r * RT
                t = ps.tile([128, RT], f32, tag="ps", name=f"psA{p}{r}")
                for (d0, d1) in [(0, 512), (512, RT)]:
                    n0, n1 = l0 + d0, l0 + d1
                    if p == "P0":
                        nc.tensor.matmul(t[:, d0:d1], wt["a1"],
                                         xa0[:, n0:n1],
                                         start=True, stop=False)
                        nc.tensor.matmul(t[:, d0:d1], wt["a2"][0:64, :],
                                         xb[0:64, n0:n1],
                                         start=False, stop=True)
                    elif p == "P1":
                        nc.tensor.matmul(t[:, d0:d1], wt["a1"],
                                         xa1[:, n0:n1],
                                         start=True, stop=False)
                        nc.tensor.matmul(t[:, d0:d1], wt["a2"][64:128, :],
                                         xb[64:128, n0:n1],
                                         start=False, stop=True,
                                         tile_position=(64, 0))
                    else:
                        nc.tensor.matmul(t[0:64, d0:d1], wt["a3"],
                                         xa0[:, n0:n1],
                                         start=True, stop=False)
                        nc.tensor.matmul(t[64:128, d0:d1], wt["a3"],
                                         xa1[:, n0:n1],
                                         start=True, stop=False,
                                         tile_position=(0, 64))
                        nc.tensor.matmul(t[:, d0:d1], wt["a4"],
                                         xb[:, n0:n1],
                                         start=False, stop=True)
                dst = xpe[p][:, 2 + l0:2 + l0 + RT]
                if r in EV_DVE:
                    nc.vector.tensor_scalar(dst, t[:], biasx[p], None,
                                            Alu.add)
                else:
                    nc.scalar.activation(dst, t[:], Act.Identity,
                                         bias=biasx[p])

            # ---------- kernel generation ----------
            kg = psk.tile([128, 512], f32, tag="kg", name="kg")
            k1ap = {0: kg[0:9, 0:192], 1: kg[0:9, 192:384]}
            k9ps = {"P0": kg[:, 384:393], "P1": kg[:, 393:402],
                    "P2": kg[:, 402:411]}

            def kg_img(i):
                pa = "P0" if i == 0 else "P1"
                k1 = k1ap[i]
                half = slice(0, 64) if i == 0 else slice(64, 128)
                nc.tensor.matmul(k1, pool16[pa][:], wka,
                                 start=True, stop=False)
                nc.tensor.matmul(k1, pool16["P2"][half, :], wkbp[half, :],
                                 start=False, stop=True)
                s = sm.tile([9, 192], f32, tag=f"sB{i}", name=f"sB{i}")
                nc.vector.tensor_tensor(s[:], k1, bkb[:], Alu.add)
                e = sm.tile([9, 192], f32, tag=f"eB{i}", name=f"eB{i}")
                nc.scalar.activation(e[:], s[:], Act.Erf, scale=INV_SQRT2)
                g = sm.tile([9, 192], f16, tag=f"gB{i}", name=f"gB{i}")
                nc.vector.scalar_tensor_tensor(g[:], e[:], 1.0, s[:],
                                               Alu.add, Alu.mult)
                g16[i] = g

            def kg_fin(p):
                kb = sm.tile([128, 9], f32, tag=f"kb{p}", name=f"kb{p}")
                ms = sm.tile([128, 1], f32, tag=f"ms{p}", name=f"ms{p}")
                nc.vector.scalar_tensor_tensor(
                    kb[:], k9ps[p], 1.0, bgb, Alu.mult, Alu.add,
                    accum_out=ms[:])
                m2 = sm.tile([128, 1], f32, tag=f"m2{p}", name=f"m2{p}")
                nc.vector.tensor_scalar(m2[:], ms[:], factor[p][:], None,
                                        Alu.mult)
                kf = sm.tile([128, 9], f32, tag=f"kfin{p}", name=f"kfin{p}")
                nc.vector.tensor_scalar(kf[:], kb[:], m2[:], None,
                                        Alu.subtract)
                kfin[p] = kf
                kn = sm.tile([128, 9], f32, tag=f"kneg{p}", name=f"kneg{p}")
                nc.vector.tensor_scalar(kn[:], kf[:], -1.0, None, Alu.mult)
                kneg[p] = kn

            def kgen0():
                kg_img(0)
                nc.tensor.matmul(k9ps["P0"], g16[0][:, 0:128], wgt[:],
                                 start=True, stop=True)
                kg_fin("P0")

            def kgen1():
                kg_img(1)
                nc.tensor.matmul(k9ps["P1"], g16[1][:, 0:128], wgt[:],
                                 start=True, stop=True)
                nc.tensor.matmul(k9ps["P2"][0:64, :], g16[0][:, 128:192],
                                 wgt[:], start=True, stop=True)
                nc.tensor.matmul(k9ps["P2"][64:128, :], g16[1][:, 128:192],
                                 wgt[:], start=True, stop=True,
                                 tile_position=(0, 64))
                kg_fin("P1")
                kg_fin("P2")

            # ---------- phase C helpers ----------
            def make_xpo(p, splits=()):
                """xpo[l] = data[l-1] over the DVE region (+1 halo row).
                Issued in row-pieces from the vector queue so each c_dve
                block only waits for its own piece."""
                xpo = xop.tile([128, XPO_SZ], f16, tag="xpo", name=f"xpo{p}")
                nc.vector.memset(xpo[:, 0:1], 0.0)
                n = (RD[p] + 1) * W + 1
                bounds = [0] + [min((s + 1) * W + 2, n) for s in splits] + [n]
                for lo_, hi_ in zip(bounds[:-1], bounds[1:]):
                    if hi_ > lo_:
                        nc.sync.dma_start(xpo[:, 1 + lo_:1 + hi_],
                                          xpe[p][:, 2 + lo_:2 + hi_])
                return xpo

            def build_diag(p):
                dd = []
                for t in range(9):
                    dtl = dg.tile([128, 128], f16, tag=f"diag{t}",
                                  name=f"diag{t}{p}")
                    nc.vector.tensor_scalar(dtl[:], ident_sb,
                                            kfin[p][:, t:t + 1], None,
                                            Alu.mult)
                    dd.append(dtl)
                diag[p] = dd

            def c_dve(p, xpo, h0, h1):
                """DVE taps for plane p over output rows [h0, h1)."""
                if h1 <= h0:
                    return
                y = y_store[p]
                kf = kfin[p]
                atp = ACT_TAPS[p]
                # center tap: y = k4 * data  (ts, 4x mode)
                nc.vector.tensor_scalar(y[:, h0 * W:h1 * W],
                                        xd(p)[:, h0 * W:h1 * W],
                                        kf[:, 4:5], None, Alu.mult)
                for b0 in range(h0, h1, TBLK):
                    b1 = min(h1, b0 + TBLK)
                    for t in [1, 7, 3, 6, 0, 2, 5, 8]:
                        dh, dw = TAPS[t]
                        r0 = max(b0, -dh)
                        r1 = min(b1, H - dh)
                        if r1 <= r0:
                            continue
                        nrow = r1 - r0
                        ts = tsp.tile([128, TBLK * W], f16, tag="tsc",
                                      name="tsc")
                        if dw == 1:
                            # row-clipped: out cols 0:95 <- src cols 1:96
                            sv = xpo[:, (r0 + dh) * W + 2:
                                     (r1 + dh) * W + 2].rearrange(
                                "c (h w) -> c h w", w=W)[:, :, 0:W - 1]
                            sc3 = ts[:, 0:nrow * W].rearrange(
                                "c (h w) -> c h w", w=W)
                            if t in atp:
                                nc.scalar.activation(sc3[:, :, 0:W - 1], sv,
                                                     Act.Copy,
                                                     scale=kf[:, t:t + 1])
                            else:
                                nc.vector.tensor_scalar(sc3[:, :, 0:W - 1],
                                                        sv, kf[:, t:t + 1],
                                                        None, Alu.mult)
                            yv = y[:, r0 * W:r1 * W].rearrange(
                                "c (h w) -> c h w", w=W)[:, :, 0:W - 1]
                            nc.vector.tensor_tensor(yv, sc3[:, :, 0:W - 1],
                                                    yv, Alu.add)
                        else:
                            n = nrow * W
                            if dw == 0:
                                src = xd(p)[:, (r0 + dh) * W:(r1 + dh) * W]
                            else:   # dw == -1, flat via xpo (garbage at col0)
                                src = xpo[:, (r0 + dh) * W:(r1 + dh) * W]
                            if t in atp:
                                nc.scalar.activation(ts[:, 0:n], src,
                                                     Act.Copy,
                                                     scale=kf[:, t:t + 1])
                            else:
                                nc.vector.tensor_scalar(ts[:, 0:n], src,
                                                        kf[:, t:t + 1],
                                                        None, Alu.mult)
                            nc.vector.tensor_tensor(y[:, r0 * W:r1 * W],
                                                    ts[:, 0:n],
                                                    y[:, r0 * W:r1 * W],
                                                    Alu.add)
                    c_fixups(p, b0, b1)

            def c_fixups(p, h0, h1):
                """Subtract wrap-column garbage for dw=-1 taps (col 0)."""
                y3 = y_store[p][:].rearrange("c (h w) -> c h w", h=H)
                xe3 = xd(p).rearrange("c (h w) -> c h w", h=H)
                kn = kneg[p]
                for t in (0, 3, 6):
                    dh, _ = TAPS[t]
                    r0 = max(h0, -dh, 1 - dh)
                    r1 = min(h1, H - dh)
                    if r1 <= r0:
                        continue
                    nc.vector.scalar_tensor_tensor(
                        y3[:, r0:r1, 0:1],
                        xe3[:, r0 + dh - 1:r1 + dh - 1, W - 1:W],
                        kn[:, t:t + 1],
                        y3[:, r0:r1, 0:1], Alu.mult, Alu.add)

            TAPORDER = [4, 1, 7, 0, 2, 3, 5, 6, 8]

            def c_pe(p):
                """PE diag-matmul taps for plane p over rows [RD[p], 96)."""
                y = y_store[p]
                dd = diag[p]
                c0 = RD[p]
                while c0 < H:
                    nrows = min(10, H - c0)
                    t_ps = ps.tile([128, RT], f32, tag="ps",
                                   name=f"psC{p}{c0}")
                    for ti, t in enumerate(TAPORDER):
                        dh, dw = TAPS[t]
                        r0 = max(c0, -dh)
                        r1 = min(c0 + nrows, H - dh)
                        if r1 <= r0:
                            continue
                        w0, w1 = ((1, W) if dw == -1 else
                                  (0, W - 1) if dw == 1 else (0, W))
                        for half in range(2):
                            hr0 = max(r0, c0 + 5 * half)
                            hr1 = min(r1, c0 + 5 * half + 5, c0 + nrows)
                            if hr1 <= hr0:
                                continue
                            sv = xpe[p][:, 2 + (hr0 + dh) * W:
                                        2 + (hr1 + dh) * W].rearrange(
                                "c (h w) -> c h w", w=W)[:, :, w0 + dw:
                                                         w1 + dw]
                            ov = t_ps[:, 512 * half:512 * half
                                      + 480].rearrange(
                                "c (h w) -> c h w", w=W)[
                                :, hr0 - (c0 + 5 * half):
                                hr1 - (c0 + 5 * half), w0:w1]
                            nc.tensor.matmul(ov, dd[t][:], sv,
                                             start=(t == 4),
                                             stop=(ti == len(TAPORDER) - 1),
                                             skip_group_check=True)
                    # evict
                    if nrows == 10:
                        src = t_ps[:, 0:1024].rearrange(
                            "c (b r) -> c b r", b=2)[:, :, 0:480]
                        nc.scalar.activation(y[:, c0 * W:(c0 + 10) * W],
                                             src, Act.Copy)
                    else:
                        for half in range(2):
                            nr = min(5, nrows - 5 * half)
                            if nr <= 0:
                                continue
                            nc.scalar.activation(
                                y[:, (c0 + 5 * half) * W:
                                  (c0 + 5 * half + nr) * W],
                                t_ps[:, 512 * half:512 * half + nr * W],
                                Act.Copy)
                    c0 += nrows

            # ---------- phase D: out = Wp y (bias folded on host) ----------
            def d_mm(o, r, t, d0, d1):
                n0, n1 = r * RT + d0, r * RT + d1
                if o == 0:
                    nc.tensor.matmul(t[:, d0:d1], wt["p1"],
                                     y_store["P0"][:, n0:n1],
                                     start=True, stop=False)
                    nc.tensor.matmul(t[:, d0:d1], wt["p2"][0:64, :],
                                     y_store["P2"][0:64, n0:n1],
                                     start=False, stop=True)
                elif o == 1:
                    nc.tensor.matmul(t[:, d0:d1], wt["p1"],
                                     y_store["P1"][:, n0:n1],
                                     start=True, stop=False)
                    nc.tensor.matmul(t[:, d0:d1], wt["p2"][64:128, :],
                                     y_store["P2"][64:128, n0:n1],
                                     start=False, stop=True,
                                     tile_position=(64, 0))
                else:
                    nc.tensor.matmul(t[0:64, d0:d1], wt["p3"],
                                     y_store["P0"][:, n0:n1],
                                     start=True, stop=False)
                    nc.tensor.matmul(t[64:128, d0:d1], wt["p3"],
                                     y_store["P1"][:, n0:n1],
                                     start=True, stop=False,
                                     tile_position=(0, 64))
                    nc.tensor.matmul(t[:, d0:d1], wt["p4"],
                                     y_store["P2"][:, n0:n1],
                                     start=False, stop=True)

            def d_group(o, r0, nr, ev="act"):
                """Output stream o for ranges [r0, r0+nr), nr in {1, 2}."""
                sta = stg.tile([128, 2048], f16, tag=f"st{o}",
                               name=f"st{o}g{r0}")
                for k in range(nr):
                    t = ps.tile([128, RT], f32, tag="ps",
                                name=f"psD{o}{r0 + k}")
                    for (d0, d1) in [(0, 512), (512, RT)]:
                        d_mm(o, r0 + k, t, d0, d1)
                    if ev == "dve":
                        nc.vector.tensor_scalar(sta[:, k * RT:(k + 1) * RT],
                                                t[:], 1.0, None, Alu.mult)
                    else:
                        nc.scalar.activation(sta[:, k * RT:(k + 1) * RT],
                                             t[:], Act.Copy)
                g0, gn = r0 * RT, nr * RT
                if o == 0:
                    nc.sync.dma_start(out0[0:128, g0:g0 + gn],
                                      sta[:, 0:gn])
                elif o == 1:
                    nc.sync.dma_start(out1[0:128, g0:g0 + gn],
                                      sta[:, 0:gn])
                else:
                    nc.gpsimd.dma_start(out0[128:192, g0:g0 + gn],
                                        sta[0:64, 0:gn])
                    nc.scalar.dma_start(out1[128:192, g0:g0 + gn],
                                        sta[64:128, 0:gn])

            def d_emit(o, ranges, ev="act", pair=True):
                i = 0
                while i < len(ranges):
                    r0 = ranges[i]
                    if (pair and i + 1 < len(ranges)
                            and ranges[i + 1] == r0 + 1):
                        d_group(o, r0, 2, ev)
                        i += 2
                    else:
                        d_group(o, r0, 1, ev)
                        i += 1

            # ================= schedule =================
            # D ranges whose y rows are entirely PE-computed go first so
            # phase D overlaps the DVE tail of phase C.
            cdiv = lambda a, b: -(-a // b)
            hi0 = cdiv(max(RD["P0"], RD["P2"]) * W, RT)
            hi1 = cdiv(max(RD["P1"], RD["P2"]) * W, RT)
            hi2 = max(hi0, hi1)

            # C planes run P2 -> P1 -> P0: P2 gates every output stream,
            # and P0's late DVE tail then gates only o0/o2 low ranges.
            pool_tree("P0", xa0, y_store["P0"][:, 0:6912],
                      after_lvl0=xa1_load)
            pool_tree("P2", xb, y_store["P0"][:, 0:6912])
            for r in range(NR):
                a_range("P0", r)
            kgen0()          # only needs pools of x, not A(P2)
            build_diag("P0")
            pool_tree("P1", xa1, xpe["P1"][:, 4:6916])
            for r in range(5):
                a_range("P2", r)
            kgen1()          # mid-A(P2): runs as soon as tree P1 lands
            for r in range(5, NR):
                a_range("P2", r)
            build_diag("P2")
            build_diag("P1")
            for r in range(NR):
                a_range("P1", r)
            # xpo transfers ride the idle sync queue; split so early
            # c_dve blocks only wait on their own rows
            xpo2 = make_xpo("P2", [33])
            xpo1 = make_xpo("P1")
            xpo0 = make_xpo("P0", [33])
            y_store["P2"] = xa0
            c_dve("P2", xpo2, 0, RD["P2"])
            c_pe("P2")
            y_store["P1"] = xa1
            c_dve("P1", xpo1, 0, RD["P1"])
            c_dve("P0", xpo0, 0, RD["P0"])
            c_pe("P0")
            # o0 hi ranges touch only PE rows of P0/P2 -> run before c_pe(P1)
            d_emit(0, list(range(hi0, NR)))
            c_pe("P1")
            d_emit(1, list(range(hi1, NR)) + list(range(0, hi1)))
            d_emit(2, list(range(hi2, NR)))
            # interleave the DVE-gated lo ranges of o0/o2 in row order
            for r in range(max(hi0, hi2)):
                if r < hi0:
                    d_group(0, r, 1)
                if r < hi2:
                    d_group(2, r, 1)

    nc.compile()
    return nc


def _get_nc():
    if "nc" not in _BUILT:
        _BUILT["nc"] = build()
    return _BUILT["nc"]


def _host_prep(Wk, bk, Wg, bg, Wx, bx, Wp, bp, dc):
    f32 = lambda a: np.ascontiguousarray(np.asarray(a, dtype=np.float32))
    f16T = lambda a: np.ascontiguousarray(
        np.asarray(a, dtype=np.float32).T.astype(np.float16))
    WxT = f16T(Wx)     # [c, o]
    WpT = f16T(Wp)
    WkT = f16T(Wk)

    def streams(WT, pre):
        blk = WT[128:192, 128:192]
        bd = np.zeros((128, 128), np.float16)
        bd[0:64, 0:64] = blk
        bd[64:128, 64:128] = blk
        tail = WT[128:192, 0:128]
        return {
            f"w_{pre}1": np.ascontiguousarray(WT[0:128, 0:128]),
            f"w_{pre}2": np.ascontiguousarray(
                np.concatenate([tail, tail], axis=0)),
            f"w_{pre}3": np.ascontiguousarray(WT[0:128, 128:192]),
            f"w_{pre}4": bd,
        }

    colv = lambda v, lo, hi: np.ascontiguousarray(
        np.asarray(v, dtype=np.float32)[lo:hi].reshape(-1, 1))
    dup_col = lambda v: np.ascontiguousarray(
        np.concatenate([colv(v, 128, 192), colv(v, 128, 192)], axis=0))

    ws = {}
    ws.update(streams(WxT, "a"))
    ws.update(streams(WpT, "p"))
    wkbp = np.concatenate([WkT[128:192], WkT[128:192]], axis=0)
    wbig = np.concatenate(
        [ws[f"w_{nm}"] for nm in ["a1", "a2", "a3", "a4",
                                  "p1", "p2", "p3", "p4"]]
        + [np.ascontiguousarray(WkT[0:128]), wkbp,
           np.eye(128, dtype=np.float16)], axis=1)
    f32bund = np.concatenate(
        [colv(bx, 0, 128), dup_col(bx), colv(dc, 0, 128), dup_col(dc),
         np.tile(f32(bg).reshape(1, 9), (128, 1))], axis=1)
    shared = {
        "wbig": np.ascontiguousarray(wbig.astype(np.float16)),
        "f32b": np.ascontiguousarray(f32bund.astype(np.float32)),
        "bk9": np.ascontiguousarray(np.tile(f32(bk).reshape(1, C), (9, 1))),
        "wg2": np.ascontiguousarray(
            (0.5 * np.asarray(Wg, dtype=np.float32)).T.astype(np.float16)),
    }
    return shared


def kernel(x, Wk, bk, Wg, bg, Wx, bx, Wp, bp, dc):
    nc = _get_nc()
    x16 = np.asarray(x, dtype=np.float32).reshape(B, C, L).astype(np.float16)
    shared = _host_prep(Wk, bk, Wg, bg, Wx, bx, Wp, bp, dc)
    in_maps = []
    for core in range(NCORES):
        m = dict(shared)
        m["x0"] = np.ascontiguousarray(x16[2 * core])
        m["x1"] = np.ascontiguousarray(x16[2 * core + 1])
        in_maps.append(m)

    res = bass_utils.run_bass_kernel_spmd(nc, in_maps,
                                          core_ids=list(range(NCORES)))
    bpc = np.asarray(bp, dtype=np.float32).reshape(1, C, 1)
    out = np.empty((B, C, H, W), dtype=np.float32)
    for core in range(NCORES):
        o0 = res.results[core]["out0"].astype(np.float32) + bpc[0]
        o1 = res.results[core]["out1"].astype(np.float32) + bpc[0]
        out[2 * core] = o0.reshape(C, H, W)
        out[2 * core + 1] = o1.reshape(C, H, W)
    return out


# revision 44
# speedup vs baseline: 1.1569x; 1.1569x over previous
"""Trainium2 Bass kernel for nn_ATConv (dynamic per-(b,c) 3x3 depthwise conv
between two 1x1 convs, with a pooled-gelu kernel-generation branch).

Sharding: data-parallel over batch B=16 across 8 NeuronCores (2 images/core).
Each core processes its 2 images as 3 "planes" of 128 partitions:
  P0 = img0 channels 0:128, P1 = img1 channels 0:128,
  P2 = packed [img0 c128:192 | img1 c128:192].

v3 design (~172us vs the 205us v2 baseline):
  - Single TileContext region, one rotating PSUM pool (3 x [128,1024] +
    a small kgen bank): all phases share PSUM, so the tile scheduler can
    overlap them freely instead of hitting pool-scope barriers.
  - Input DMA is piece-streamed (xa0 on sync in 6 pieces, xb on gpsimd,
    xa1 queued behind both) and phase A consumes it per-range, so the
    8us DMA head and ~25us load window overlap the first compute.
  - Weights ride the scalar queue in parallel with the first x pieces.
  - Pooling for kernel-gen is a fp16 tree-reduce on DVE (4 halvings +
    tensor_reduce), ~6us/plane instead of ~11us of 1x copies.
  - kgen0 needs only pools of x -> runs before A(P2); kgen1 needs only
    pools + g16[0] -> emitted mid-A(P2).  Both kernel-gen chains use
    spare columns of a dedicated 1-bank PSUM tile.
  - Phase C is split DVE-rows/PE-rows per plane (RD):  DVE taps use
    tensor_scalar 4x + tensor_tensor 2x with an element-shifted copy
    (xpo) for odd column shifts; dw=+1 taps use row-clipped 2D APs (no
    fixup), dw=-1 taps keep flat reads + tiny column fixups.  PE taps
    are diagonal matmuls with row/col-clipped 2D moving APs (fixup
    free), evicted by ACT.
  - C planes run P2 -> P1 -> P0 on DVE (P2 gates every output stream);
    phase D output streams are emitted at their earliest unlock points
    (o0 hi-ranges between c_pe(P0) and c_pe(P1), etc.) with hi rows
    first so D overlaps the DVE tail.
  - xpo copies ride the idle sync queue in row-pieces so c_dve blocks
    only wait for their own rows.
  - Output stores are 2-range (4KB/row) DMAs spread over sync, gpsimd
    and scalar queues.
"""
import numpy as np

import concourse.bacc as bacc
import concourse.mybir as mybir
import concourse.tile as tile
from concourse import bass_utils

dt = mybir.dt
Alu = mybir.AluOpType
Act = mybir.ActivationFunctionType
Ax = mybir.AxisListType

B, C, H, W = 16, 192, 96, 96
L = H * W            # 9216
K2 = 9
SEG = L // K2        # 1024
NCORES = 8
NR = 9               # ranges of SEG
RT = 1024
INV_SQRT2 = float(1.0 / np.sqrt(2.0))

# tap index t = 3*(dh+1) + (dw+1); center tap = 4
TAPS = [(t // 3 - 1, t % 3 - 1) for t in range(9)]
# rows [0, RD[p]) of plane p run on DVE; the rest on PE (diag matmuls)
RD = {"P0": 48, "P1": 24, "P2": 50}
RDMAX = max(RD.values())
XPO_SZ = (RDMAX + 1) * W + 2
# ranges whose phase-A eviction runs on DVE instead of ACT (per plane)
EV_DVE = set()
# taps whose scale-copy runs on ACT (per plane); adds stay on DVE
ACT_TAPS = {"P0": (), "P1": (), "P2": ()}
# DVE tap row-block size (bounds the tap scratch tile)
TBLK = 33

_BUILT = {}


def build():
    nc = bacc.Bacc("TRN2", target_bir_lowering=False, debug=False,
                   num_devices=NCORES)

    f16, f32 = dt.float16, dt.float32
    x0 = nc.dram_tensor("x0", [C, L], f16, kind="ExternalInput").ap()
    x1 = nc.dram_tensor("x1", [C, L], f16, kind="ExternalInput").ap()
    wbig = nc.dram_tensor("wbig", [128, 1408], f16, kind="ExternalInput").ap()
    f32b = nc.dram_tensor("f32b", [128, 13], f32, kind="ExternalInput").ap()
    bk9 = nc.dram_tensor("bk9", [9, 192], f32, kind="ExternalInput").ap()
    wg2 = nc.dram_tensor("wg2", [9, 9], f16, kind="ExternalInput").ap()
    out0 = nc.dram_tensor("out0", [C, L], f16, kind="ExternalOutput").ap()
    out1 = nc.dram_tensor("out1", [C, L], f16, kind="ExternalOutput").ap()

    PL = ["P0", "P1", "P2"]

    with tile.TileContext(nc) as tc:
        with tc.tile_pool(name="wpool", bufs=1) as wp, \
             tc.tile_pool(name="xin", bufs=1) as xin, \
             tc.tile_pool(name="small", bufs=1) as sm, \
             tc.tile_pool(name="xppool", bufs=1) as xpp, \
             tc.tile_pool(name="ypool", bufs=1) as yp, \
             tc.tile_pool(name="xopool", bufs=2) as xop, \
             tc.tile_pool(name="tspool", bufs=2) as tsp, \
             tc.tile_pool(name="dg", bufs=2) as dg, \
             tc.tile_pool(name="stage", bufs=3) as stg, \
             tc.tile_pool(name="ps", bufs=3, space="PSUM") as ps, \
             tc.tile_pool(name="psk", bufs=1, space="PSUM") as psk:

            # ---- persistent weights / biases (front of the sync queue) ----
            wbig_t = wp.tile([128, 1408], f16, tag="wbig", name="wbig_t")
            nc.scalar.dma_start(wbig_t[:], wbig[:, :])
            f32b_t = wp.tile([128, 13], f32, tag="f32b", name="f32b_t")
            nc.scalar.dma_start(f32b_t[:], f32b[:, :])
            bkb = wp.tile([9, 192], f32, tag="bkb", name="bkb")
            nc.scalar.dma_start(bkb[:], bk9[:, :])
            wgt = wp.tile([9, 9], f16, tag="wgt", name="wgt")
            nc.scalar.dma_start(wgt[:], wg2[:, :])

            # ---- input x tiles ----
            # xa0 (sync) and xb (gpsimd) stream first -- they feed phase
            # A(P0).  xa1's DMAs are issued from the vector queue, gated
            # behind the first pool-tree op, so its transfers don't steal
            # bandwidth from the critical xa0/xb head.
            xa0 = xin.tile([128, L], f16, tag="xa0", name="xa0")
            xa1 = xin.tile([128, L], f16, tag="xa1", name="xa1")
            xb = xin.tile([128, L], f16, tag="xb", name="xb")
            Q = L // 3
            for lo_, hi_ in [(0, 1024), (1024, 2048), (2048, 3072),
                             (3072, 4608), (4608, 6656), (6656, L)]:
                nc.sync.dma_start(xa0[:, lo_:hi_], x0[0:128, lo_:hi_])
            for qi in range(3):
                lo_, hi_ = qi * Q, (qi + 1) * Q
                nc.gpsimd.dma_start(xb[0:64, lo_:hi_], x0[128:192, lo_:hi_])
                nc.gpsimd.dma_start(xb[64:128, lo_:hi_], x1[128:192, lo_:hi_])

            def xa1_load(qi):
                # behind xa0 on sync / behind xb on gpsimd, so the xa1
                # transfers start only once the phase-A(P0) inputs are in
                lo_, hi_ = qi * Q, (qi + 1) * Q
                eng = nc.sync if qi == 0 else nc.gpsimd
                eng.dma_start(xa1[:, lo_:hi_], x1[0:128, lo_:hi_])
            wt = {}
            _off = 0
            for nm, cols in [("a1", 128), ("a2", 128), ("a3", 64),
                             ("a4", 128), ("p1", 128), ("p2", 128),
                             ("p3", 64), ("p4", 128)]:
                wt[nm] = wbig_t[:, _off:_off + cols]
                _off += cols
            wka = wbig_t[:, _off:_off + 192]; _off += 192
            wkbp = wbig_t[:, _off:_off + 192]; _off += 192
            ident_sb = wbig_t[:, _off:_off + 128]; _off += 128
            bias = {"bx_a": f32b_t[:, 0:1], "bx_b": f32b_t[:, 1:2],
                    "dc_a": f32b_t[:, 2:3], "dc_b": f32b_t[:, 3:4]}
            bgb = f32b_t[:, 4:13]
            biasx = {"P0": bias["bx_a"], "P1": bias["bx_a"],
                     "P2": bias["bx_b"]}

            factor = {}
            for p, srcn in [("P0", "dc_a"), ("P2", "dc_b")]:
                f = sm.tile([128, 1], f32, tag=f"factor{p}", name=f"factor{p}")
                nc.scalar.activation(f[:], bias[srcn], Act.Sigmoid,
                                     scale=1.0, bias=0.0)
                f9 = sm.tile([128, 1], f32, tag=f"f9{p}", name=f"f9{p}")
                nc.vector.tensor_scalar(f9[:], f[:], 1.0 / 9, None, Alu.mult)
                factor[p] = f9
            factor["P1"] = factor["P0"]

            # xpe tiles carry 2 zero guard elements on each side (data at
            # offset 2) so shifted reads at the plane edges see zeros.
            xpe = {p: xpp.tile([128, L + 4], f16, tag=f"xpe{p}",
                               name=f"xpe{p}")
                   for p in PL}
            for p in PL:
                nc.vector.memset(xpe[p][:, 0:2], 0.0)
                nc.vector.memset(xpe[p][:, L + 2:L + 4], 0.0)

            pool = {p: sm.tile([128, 9], f32, tag=f"pool{p}",
                               name=f"pool{p}")
                    for p in PL}
            pool16 = {p: sm.tile([128, 9], f16, tag=f"pool16{p}",
                                 name=f"pool16{p}")
                      for p in PL}

            y_store = {"P0": yp.tile([128, L], f16, tag="yP0", name="yP0")}

            kfin = {}
            kneg = {}
            g16 = {}
            diag = {}

            def xd(p):
                return xpe[p][:, 2:2 + L]

            # ---------- pooling: fp16 tree reduce on DVE ----------
            # scratch: P0/P2 borrow yP0 (only written later, in phase C);
            # P1 borrows xpe[P1] (written later by A(P1) evictions).
            def pool_tree(p, xt, scr_base, after_lvl0=None):
                s1 = scr_base[:, 0:4608]
                s2 = scr_base[:, 4608:6912]
                for qi in range(3):
                    src = xt[:, qi * 3072:(qi + 1) * 3072].rearrange(
                        "c (s h) -> c s h", s=3)
                    d = s1[:, qi * 1536:(qi + 1) * 1536].rearrange(
                        "c (s h) -> c s h", s=3)
                    nc.vector.tensor_tensor(d, src[:, :, 0:512],
                                            src[:, :, 512:1024], Alu.add)
                    if after_lvl0 is not None:
                        after_lvl0(qi)
                s1v = s1.rearrange("c (s h) -> c s h", s=9)      # [c,9,512]
                s2v = s2.rearrange("c (s h) -> c s h", s=9)      # [c,9,256]
                nc.vector.tensor_tensor(s2v, s1v[:, :, 0:256],
                                        s1v[:, :, 256:512], Alu.add)
                d2 = s1[:, 0:1152].rearrange("c (s h) -> c s h", s=9)
                nc.vector.tensor_tensor(d2, s2v[:, :, 0:128],
                                        s2v[:, :, 128:256], Alu.add)
                d3 = s2[:, 0:576].rearrange("c (s h) -> c s h", s=9)
                nc.vector.tensor_tensor(d3, d2[:, :, 0:64],
                                        d2[:, :, 64:128], Alu.add)
                nc.vector.tensor_reduce(
                    pool[p][:].rearrange("c (s o) -> c s o", s=9), d3,
                    Ax.X, Alu.add)
                nc.vector.tensor_scalar(pool16[p][:], pool[p][:],
                                        1.0 / SEG, None, Alu.mult)

            # ---------- phase A: xp = Wx x + bx ----------
            def a_range(p, r):
                l0 = r * RT
                t = ps.tile([128, RT], f32, tag="ps", name=f"psA{p}{r}")
                for (d0, d1) in [(0, 512), (512, RT)]:
                    n0, n1 = l0 + d0, l0 + d1
                    if p == "P0":
                        nc.tensor.matmul(t[:, d0:d1], wt["a1"],
                                         xa0[:, n0:n1],
                                         start=True, stop=False)
                        nc.tensor.matmul(t[:, d0:d1], wt["a2"][0:64, :],
                                         xb[0:64, n0:n1],
                                         start=False, stop=True)
                    elif p == "P1":
                        nc.tensor.matmul(t[:, d0:d1], wt["a1"],
                                         xa1[:, n0:n1],
                                         start=True, stop=False)
                        nc.tensor.matmul(t[:, d0:d1], wt["a2"][64:128, :],
                                         xb[64:128, n0:n1],
                                         start=False, stop=True,
                                         tile_position=(64, 0))
                    else:
                        nc.tensor.matmul(t[0:64, d0:d1], wt["a3"],
                                         xa0[:, n0:n1],
                                         start=True, stop=False)
                        nc.tensor.matmul(t[64:128, d0:d1], wt["a3"],
                                         xa1[:, n0:n1],
                                         start=True, stop=False,
                                         tile_position=(0, 64))
                        nc.tensor.matmul(t[:, d0:d1], wt["a4"],
                                         xb[:, n0:n1],
                                         start=False, stop=True)
                dst = xpe[p][:, 2 + l0:2 + l0 + RT]
                if r in EV_DVE:
                    nc.vector.tensor_scalar(dst, t[:], biasx[p], None,
                                            Alu.add)
                else:
                    nc.scalar.activation(dst, t[:], Act.Identity,
                                         bias=biasx[p])

            # ---------- kernel generation ----------
            kg = psk.tile([128, 512], f32, tag="kg", name="kg")
            k1ap = {0: kg[0:9, 0:192], 1: kg[0:9, 192:384]}
            k9ps = {"P0": kg[:, 384:393], "P1": kg[:, 393:402],
                    "P2": kg[:, 402:411]}

            def kg_img(i):
                pa = "P0" if i == 0 else "P1"
                k1 = k1ap[i]
                half = slice(0, 64) if i == 0 else slice(64, 128)
                nc.tensor.matmul(k1, pool16[pa][:], wka,
                                 start=True, stop=False)
                nc.tensor.matmul(k1, pool16["P2"][half, :], wkbp[half, :],
                                 start=False, stop=True)
                s = sm.tile([9, 192], f32, tag=f"sB{i}", name=f"sB{i}")
                nc.vector.tensor_tensor(s[:], k1, bkb[:], Alu.add)
                e = sm.tile([9, 192], f32, tag=f"eB{i}", name=f"eB{i}")
                nc.scalar.activation(e[:], s[:], Act.Erf, scale=INV_SQRT2)
                g = sm.tile([9, 192], f16, tag=f"gB{i}", name=f"gB{i}")
                nc.vector.scalar_tensor_tensor(g[:], e[:], 1.0, s[:],
                                               Alu.add, Alu.mult)
                g16[i] = g

            def kg_fin(p):
                kb = sm.tile([128, 9], f32, tag=f"kb{p}", name=f"kb{p}")
                ms = sm.tile([128, 1], f32, tag=f"ms{p}", name=f"ms{p}")
                nc.vector.scalar_tensor_tensor(
                    kb[:], k9ps[p], 1.0, bgb, Alu.mult, Alu.add,
                    accum_out=ms[:])
                m2 = sm.tile([128, 1], f32, tag=f"m2{p}", name=f"m2{p}")
                nc.vector.tensor_scalar(m2[:], ms[:], factor[p][:], None,
                                        Alu.mult)
                kf = sm.tile([128, 9], f32, tag=f"kfin{p}", name=f"kfin{p}")
                nc.vector.tensor_scalar(kf[:], kb[:], m2[:], None,
                                        Alu.subtract)
                kfin[p] = kf
                kn = sm.tile([128, 9], f32, tag=f"kneg{p}", name=f"kneg{p}")
                nc.vector.tensor_scalar(kn[:], kf[:], -1.0, None, Alu.mult)
                kneg[p] = kn

            def kgen0():
                kg_img(0)
                nc.tensor.matmul(k9ps["P0"], g16[0][:, 0:128], wgt[:],
                                 start=True, stop=True)
                kg_fin("P0")

            def kgen1():
                kg_img(1)
                nc.tensor.matmul(k9ps["P1"], g16[1][:, 0:128], wgt[:],
                                 start=True, stop=True)
                nc.tensor.matmul(k9ps["P2"][0:64, :], g16[0][:, 128:192],
                                 wgt[:], start=True, stop=True)
                nc.tensor.matmul(k9ps["P2"][64:128, :], g16[1][:, 128:192],
                                 wgt[:], start=True, stop=True,
                                 tile_position=(0, 64))
                kg_fin("P1")
                kg_fin("P2")

            # ---------- phase C helpers ----------
            def make_xpo(p, splits=()):
                """xpo[l] = data[l-1] over the DVE region (+1 halo row).
                Issued in row-pieces from the vector queue so each c_dve
                block only waits for its own piece."""
                xpo = xop.tile([128, XPO_SZ], f16, tag="xpo", name=f"xpo{p}")
                nc.vector.memset(xpo[:, 0:1], 0.0)
                n = (RD[p] + 1) * W + 1
                bounds = [0] + [min((s + 1) * W + 2, n) for s in splits] + [n]
                for lo_, hi_ in zip(bounds[:-1], bounds[1:]):
                    if hi_ > lo_:
                        nc.sync.dma_start(xpo[:, 1 + lo_:1 + hi_],
                                          xpe[p][:, 2 + lo_:2 + hi_])
                return xpo

            def build_diag(p):
                dd = []
                for t in range(9):
                    dtl = dg.tile([128, 128], f16, tag=f"diag{t}",
                                  name=f"diag{t}{p}")
                    nc.vector.tensor_scalar(dtl[:], ident_sb,
                                            kfin[p][:, t:t + 1], None,
                                            Alu.mult)
                    dd.append(dtl)
                diag[p] = dd

            def c_dve(p, xpo, h0, h1):
                """DVE taps for plane p over output rows [h0, h1)."""
                if h1 <= h0:
                    return
                y = y_store[p]
                kf = kfin[p]
                atp = ACT_TAPS[p]
                # center tap: y = k4 * data  (ts, 4x mode)
                nc.vector.tensor_scalar(y[:, h0 * W:h1 * W],
                                        xd(p)[:, h0 * W:h1 * W],
                                        kf[:, 4:5], None, Alu.mult)
                for b0 in range(h0, h1, TBLK):
                    b1 = min(h1, b0 + TBLK)
                    for t in [1, 7, 3, 6, 0, 2, 5, 8]:
                        dh, dw = TAPS[t]
                        r0 = max(b0, -dh)
                        r1 = min(b1, H - dh)
                        if r1 <= r0:
                            continue
                        nrow = r1 - r0
                        ts = tsp.tile([128, TBLK * W], f16, tag="tsc",
                                      name="tsc")
                        if dw == 1:
                            # row-clipped: out cols 0:95 <- src cols 1:96
                            sv = xpo[:, (r0 + dh) * W + 2:
                                     (r1 + dh) * W + 2].rearrange(
                                "c (h w) -> c h w", w=W)[:, :, 0:W - 1]
                            sc3 = ts[:, 0:nrow * W].rearrange(
                                "c (h w) -> c h w", w=W)
                            if t in atp:
                                nc.scalar.activation(sc3[:, :, 0:W - 1], sv,
                                                     Act.Copy,
                                                     scale=kf[:, t:t + 1])
                            else:
                                nc.vector.tensor_scalar(sc3[:, :, 0:W - 1],
                                                        sv, kf[:, t:t + 1],
                                                        None, Alu.mult)
                            yv = y[:, r0 * W:r1 * W].rearrange(
                                "c (h w) -> c h w", w=W)[:, :, 0:W - 1]
                            nc.vector.tensor_tensor(yv, sc3[:, :, 0:W - 1],
                                                    yv, Alu.add)
                        else:
                            n = nrow * W
                            if dw == 0:
                                src = xd(p)[:, (r0 + dh) * W:(r1 + dh) * W]
                            else:   # dw == -1, flat via xpo (garbage at col0)
                                src = xpo[:, (r0 + dh) * W:(r1 + dh) * W]
                            if t in atp:
                                nc.scalar.activation(ts[:, 0:n], src,
                                                     Act.Copy,
                                                     scale=kf[:, t:t + 1])
                            else:
                                nc.vector.tensor_scalar(ts[:, 0:n], src,
                                                        kf[:, t:t + 1],
                                                        None, Alu.mult)
                            nc.vector.tensor_tensor(y[:, r0 * W:r1 * W],
                                                    ts[:, 0:n],
                                                    y[:, r0 * W:r1 * W],
                                                    Alu.add)
                    c_fixups(p, b0, b1)

            def c_fixups(p, h0, h1):
                """Subtract wrap-column garbage for dw=-1 taps (col 0)."""
                y3 = y_store[p][:].rearrange("c (h w) -> c h w", h=H)
                xe3 = xd(p).rearrange("c (h w) -> c h w", h=H)
                kn = kneg[p]
                for t in (0, 3, 6):
                    dh, _ = TAPS[t]
                    r0 = max(h0, -dh, 1 - dh)
                    r1 = min(h1, H - dh)
                    if r1 <= r0:
                        continue
                    nc.vector.scalar_tensor_tensor(
                        y3[:, r0:r1, 0:1],
                        xe3[:, r0 + dh - 1:r1 + dh - 1, W - 1:W],
                        kn[:, t:t + 1],
                        y3[:, r0:r1, 0:1], Alu.mult, Alu.add)

            TAPORDER = [4, 1, 7, 0, 2, 3, 5, 6, 8]

            def c_pe(p):
                """PE diag-matmul taps for plane p over rows [RD[p], 96)."""
                y = y_store[p]
                dd = diag[p]
                c0 = RD[p]
                while c0 < H:
                    nrows = min(10, H - c0)
                    t_ps = ps.tile([128, RT], f32, tag="ps",
                                   name=f"psC{p}{c0}")
                    for ti, t in enumerate(TAPORDER):
                        dh, dw = TAPS[t]
                        r0 = max(c0, -dh)
                        r1 = min(c0 + nrows, H - dh)
                        if r1 <= r0:
                            continue
                        w0, w1 = ((1, W) if dw == -1 else
                                  (0, W - 1) if dw == 1 else (0, W))
                        for half in range(2):
                            hr0 = max(r0, c0 + 5 * half)
                            hr1 = min(r1, c0 + 5 * half + 5, c0 + nrows)
                            if hr1 <= hr0:
                                continue
                            sv = xpe[p][:, 2 + (hr0 + dh) * W:
                                        2 + (hr1 + dh) * W].rearrange(
                                "c (h w) -> c h w", w=W)[:, :, w0 + dw:
                                                         w1 + dw]
                            ov = t_ps[:, 512 * half:512 * half
                                      + 480].rearrange(
                                "c (h w) -> c h w", w=W)[
                                :, hr0 - (c0 + 5 * half):
                                hr1 - (c0 + 5 * half), w0:w1]
                            nc.tensor.matmul(ov, dd[t][:], sv,
                                             start=(t == 4),
                                             stop=(ti == len(TAPORDER) - 1),
                                             skip_group_check=True)
                    # evict
                    if nrows == 10:
                        src = t_ps[:, 0:1024].rearrange(
                            "c (b r) -> c b r", b=2)[:, :, 0:480]
                        nc.scalar.activation(y[:, c0 * W:(c0 + 10) * W],
                                             src, Act.Copy)
                    else:
                        for half in range(2):
                            nr = min(5, nrows - 5 * half)
                            if nr <= 0:
                                continue
                            nc.scalar.activation(
                                y[:, (c0 + 5 * half) * W:
                                  (c0 + 5 * half + nr) * W],
                                t_ps[:, 512 * half:512 * half + nr * W],
                                Act.Copy)
                    c0 += nrows

            # ---------- phase D: out = Wp y (bias folded on host) ----------
            def d_mm(o, r, t, d0, d1):
                n0, n1 = r * RT + d0, r * RT + d1
                if o == 0:
                    nc.tensor.matmul(t[:, d0:d1], wt["p1"],
                                     y_store["P0"][:, n0:n1],
                                     start=True, stop=False)
                    nc.tensor.matmul(t[:, d0:d1], wt["p2"][0:64, :],
                                     y_store["P2"][0:64, n0:n1],
                                     start=False, stop=True)
                elif o == 1:
                    nc.tensor.matmul(t[:, d0:d1], wt["p1"],
                                     y_store["P1"][:, n0:n1],
                                     start=True, stop=False)
                    nc.tensor.matmul(t[:, d0:d1], wt["p2"][64:128, :],
                                     y_store["P2"][64:128, n0:n1],
                                     start=False, stop=True,
                                     tile_position=(64, 0))
                else:
                    nc.tensor.matmul(t[0:64, d0:d1], wt["p3"],
                                     y_store["P0"][:, n0:n1],
                                     start=True, stop=False)
                    nc.tensor.matmul(t[64:128, d0:d1], wt["p3"],
                                     y_store["P1"][:, n0:n1],
                                     start=True, stop=False,
                                     tile_position=(0, 64))
                    nc.tensor.matmul(t[:, d0:d1], wt["p4"],
                                     y_store["P2"][:, n0:n1],
                                     start=False, stop=True)

            def d_group(o, r0, nr, ev="act"):
                """Output stream o for ranges [r0, r0+nr), nr in {1, 2}."""
                sta = stg.tile([128, 2048], f16, tag=f"st{o}",
                               name=f"st{o}g{r0}")
                for k in range(nr):
                    t = ps.tile([128, RT], f32, tag="ps",
                                name=f"psD{o}{r0 + k}")
                    for (d0, d1) in [(0, 512), (512, RT)]:
                        d_mm(o, r0 + k, t, d0, d1)
                    if ev == "dve":
                        nc.vector.tensor_scalar(sta[:, k * RT:(k + 1) * RT],
                                                t[:], 1.0, None, Alu.mult)
                    else:
                        nc.scalar.activation(sta[:, k * RT:(k + 1) * RT],
                                             t[:], Act.Copy)
                g0, gn = r0 * RT, nr * RT
                if o == 0:
                    nc.sync.dma_start(out0[0:128, g0:g0 + gn],
                                      sta[:, 0:gn])
                elif o == 1:
                    nc.sync.dma_start(out1[0:128, g0:g0 + gn],
                                      sta[:, 0:gn])
                else:
                    nc.gpsimd.dma_start(out0[128:192, g0:g0 + gn],
                                        sta[0:64, 0:gn])
                    nc.scalar.dma_start(out1[128:192, g0:g0 + gn],
                                        sta[64:128, 0:gn])

            def d_emit(o, ranges, ev="act", pair=True):
                i = 0
                while i < len(ranges):
                    r0 = ranges[i]
                    if (pair and i + 1 < len(ranges)
                            and ranges[i + 1] == r0 + 1):
                        d_group(o, r0, 2, ev)
                        i += 2
                    else:
                        d_group(o, r0, 1, ev)
                        i += 1

            # ================= schedule =================
            # D ranges whose y rows are entirely PE-computed go first so
            # phase D overlaps the DVE tail of phase C.
            cdiv = lambda a, b: -(-a // b)
            hi0 = cdiv(max(RD["P0"], RD["P2"]) * W, RT)
            hi1 = cdiv(max(RD["P1"], RD["P2"]) * W, RT)
            hi2 = max(hi0, hi1)

            # C planes run P2 -> P1 -> P0: P2 gates every output stream,
            # and P0's late DVE tail then gates only o0/o2 low ranges.
            pool_tree("P0", xa0, y_store["P0"][:, 0:6912],
                      after_lvl0=xa1_load)
            pool_tree("P2", xb, y_store["P0"][:, 0:6912])
            for r in range(NR):
                a_range("P0", r)
            kgen0()          # only needs pools of x, not A(P2)
            build_diag("P0")
            pool_tree("P1", xa1, xpe["P1"][:, 4:6916])
            for r in range(5):
                a_range("P2", r)
            kgen1()          # mid-A(P2): runs as soon as tree P1 lands
            for r in range(5, NR):
                a_range("P2", r)
            build_diag("P2")
            build_diag("P1")
            for r in range(NR):
                a_range("P1", r)
            # xpo transfers ride the idle sync queue; split so early
            # c_dve blocks only wait on their own rows
            xpo2 = make_xpo("P2", [33])
            xpo1 = make_xpo("P1")
            xpo0 = make_xpo("P0", [33])
            y_store["P2"] = xa0
            c_dve("P2", xpo2, 0, RD["P2"])
            c_pe("P2")
            y_store["P1"] = xa1
            c_dve("P1", xpo1, 0, RD["P1"])
            c_dve("P0", xpo0, 0, RD["P0"])
            c_pe("P0")
            # o0 hi ranges touch only PE rows of P0/P2 -> run before c_pe(P1)
            d_emit(0, list(range(hi0, NR)))
            c_pe("P1")
            d_emit(1, list(range(hi1, NR)) + list(range(0, hi1)))
            d_emit(2, list(range(hi2, NR)))
            d_emit(0, list(range(0, hi0)), pair=False)
            d_emit(2, list(range(0, hi2)), pair=False)

    nc.compile()
    return nc


def _get_nc():
    if "nc" not in _BUILT:
        _BUILT["nc"] = build()
    return _BUILT["nc"]


def _host_prep(Wk, bk, Wg, bg, Wx, bx, Wp, bp, dc):
    f32 = lambda a: np.ascontiguousarray(np.asarray(a, dtype=np.float32))
    f16T = lambda a: np.ascontiguousarray(
        np.asarray(a, dtype=np.float32).T.astype(np.float16))
    WxT = f16T(Wx)     # [c, o]
    WpT = f16T(Wp)
    WkT = f16T(Wk)

    def streams(WT, pre):
        blk = WT[128:192, 128:192]
        bd = np.zeros((128, 128), np.float16)
        bd[0:64, 0:64] = blk
        bd[64:128, 64:128] = blk
        tail = WT[128:192, 0:128]
        return {
            f"w_{pre}1": np.ascontiguousarray(WT[0:128, 0:128]),
            f"w_{pre}2": np.ascontiguousarray(
                np.concatenate([tail, tail], axis=0)),
            f"w_{pre}3": np.ascontiguousarray(WT[0:128, 128:192]),
            f"w_{pre}4": bd,
        }

    colv = lambda v, lo, hi: np.ascontiguousarray(
        np.asarray(v, dtype=np.float32)[lo:hi].reshape(-1, 1))
    dup_col = lambda v: np.ascontiguousarray(
        np.concatenate([colv(v, 128, 192), colv(v, 128, 192)], axis=0))

    ws = {}
    ws.update(streams(WxT, "a"))
    ws.update(streams(WpT, "p"))
    wkbp = np.concatenate([WkT[128:192], WkT[128:192]], axis=0)
    wbig = np.concatenate(
        [ws[f"w_{nm}"] for nm in ["a1", "a2", "a3", "a4",
                                  "p1", "p2", "p3", "p4"]]
        + [np.ascontiguousarray(WkT[0:128]), wkbp,
           np.eye(128, dtype=np.float16)], axis=1)
    f32bund = np.concatenate(
        [colv(bx, 0, 128), dup_col(bx), colv(dc, 0, 128), dup_col(dc),
         np.tile(f32(bg).reshape(1, 9), (128, 1))], axis=1)
    shared = {
        "wbig": np.ascontiguousarray(wbig.astype(np.float16)),
        "f32b": np.ascontiguousarray(f32bund.astype(np.float32)),
        "bk9": np.ascontiguousarray(np.tile(f32(bk).reshape(1, C), (9, 1))),
        "wg2": np.ascontiguousarray(
            (0.5 * np.asarray(Wg, dtype=np.float32)).T.astype(np.float16)),
    }
    return shared


def kernel(x, Wk, bk, Wg, bg, Wx, bx, Wp, bp, dc):
    nc = _get_nc()
    x16 = np.asarray(x, dtype=np.float32).reshape(B, C, L).astype(np.float16)
    shared = _host_prep(Wk, bk, Wg, bg, Wx, bx, Wp, bp, dc)
    in_maps = []
    for core in range(NCORES):
        m = dict(shared)
        m["x0"] = np.ascontiguousarray(x16[2 * core])
        m["x1"] = np.ascontiguousarray(x16[2 * core + 1])
        in_maps.append(m)

    res = bass_utils.run_bass_kernel_spmd(nc, in_maps,
                                          core_ids=list(range(NCORES)))
    bpc = np.asarray(bp, dtype=np.float32).reshape(1, C, 1)
    out = np.empty((B, C, H, W), dtype=np.float32)
    for core in range(NCORES):
        o0 = res.results[core]["out0"].astype(np.float32) + bpc[0]
        o1 = res.results[core]["out1"].astype(np.float32) + bpc[0]
        out[2 * core] = o0.reshape(C, H, W)
        out[2 * core + 1] = o1.reshape(C, H, W)
    return out
